# revision 29
# baseline (speedup 1.0000x reference)
"""GRU encoder step (embedding lookup + GRUCell, batch=1) on 8 TRN2 cores.

Sharding: each core k computes hidden dims [32k, 32k+32) of h_new.

Per-core packed operand layout (HID=256, G=HID//8=32, WA=515):

  a_mat [128, 515] f32 (per-core):
     partitions  0:32  -> gate r rows   cols: [w_ih[S] | w_hh[S] | b_ih[S], b_hh[S] | hs]
     partitions 32:64  -> i_n rows      [w_ih[512+S] | 0 | b_ih, 0 | 0]
     partitions 64:96  -> gate z rows   (w_ih/w_hh row offset 256)
     partitions 96:128 -> h_n rows      [0 | w_hh[512+S] | b_hh, 0 | 0]
     col 514 partitions 0:32 = hs (this core's h slice, used in the final blend)
  bh_mat [128, 258] f32 (shared): cols 0:256 = h row, 256:258 = 1.0
  idx    [96, 1] i32 (shared): token index replicated
  table  [100000, 256] f32 (shared): full embedding

Device program (straight-line raw bass, manual semaphores, no collectives):
  sync   : idx/a/bh DMAs in; result DMA out                   (HWDGE)
  gpsimd : two indirect gathers bx[p,:] = table[idx,:]        (SWDGE)
           first partitions 0:64 (r,i_n — the critical chain),
           then 64:96 (z — only needed for the final blend)
  vector : gh = sum(A[:,256:514]*Bh) ; ghn0 = copy gh[96:128]
           gx[0:64] ; gx[64:96] (fused mul+reduce per gather half)
           d = hs-n ; out = d*z + n (wide 512B/partition write)
  scalar : r = sigmoid(gx+gh) ; p = ghn0*r ; i_n0 = gx+gh ;
           n = tanh(p + bias=i_n0) ; z = sigmoid(gx+gh)
"""

import os
import sys

import numpy as np

for _p in ("/opt/trn_rl_repo",):
    if _p not in sys.path and os.path.isdir(_p):
        sys.path.insert(0, _p)

import concourse.bass as bass
from concourse import mybir

VOCAB = 100000
HID = 256
NCORES = 8
G = HID // NCORES  # 32
WA = 2 * HID + 3  # 515: [x-weights 256 | h-weights 256 | bias0, bias1 | hs]

_cached = None


def build_program():
    nc = bass.Bass(
        "TRN2",
        target_bir_lowering=False,
        debug=False,
        enable_asserts=True,
        num_devices=NCORES,
    )
    f32 = mybir.dt.float32
    i32 = mybir.dt.int32

    table = nc.dram_tensor("table", [VOCAB, HID], f32, kind="ExternalInput").ap()
    a_d = nc.dram_tensor("a_mat", [128, WA], f32, kind="ExternalInput").ap()
    bh_d = nc.dram_tensor("bh_mat", [128, HID + 2], f32, kind="ExternalInput").ap()
    idx_d = nc.dram_tensor("idx", [3 * G, 1], i32, kind="ExternalInput").ap()
    # [G, 128] so each partition's DMA write is 512B (avoids sub-512B RMW);
    # host reads column 0.
    out_d = nc.dram_tensor("out", [G, 128], f32, kind="ExternalOutput").ap()

    GP = 3 * G  # 96 partitions carry x-side rows (r, z, i_n)
    a_sb = nc.alloc_sbuf_tensor("a_sb", [128, WA], f32).ap()
    bh_sb = nc.alloc_sbuf_tensor("bh_sb", [128, HID + 2], f32).ap()
    bx_sb = nc.alloc_sbuf_tensor("bx_sb", [GP, HID], f32).ap()
    idx_sb = nc.alloc_sbuf_tensor("idx_sb", [GP, 1], i32).ap()
    s1 = nc.alloc_sbuf_tensor("s1", [GP, HID], f32).ap()
    s2 = nc.alloc_sbuf_tensor("s2", [128, HID + 2], f32).ap()
    gh = nc.alloc_sbuf_tensor("gh", [128, 1], f32).ap()
    gx = nc.alloc_sbuf_tensor("gx", [GP, 1], f32).ap()
    r_t = nc.alloc_sbuf_tensor("r_t", [G, 1], f32).ap()
    z_t = nc.alloc_sbuf_tensor("z_t", [G, 1], f32).ap()
    in0_t = nc.alloc_sbuf_tensor("in0_t", [G, 1], f32).ap()
    ghn0 = nc.alloc_sbuf_tensor("ghn0", [G, 1], f32).ap()
    p_t = nc.alloc_sbuf_tensor("p_t", [G, 1], f32).ap()
    n_t = nc.alloc_sbuf_tensor("n_t", [G, 1], f32).ap()
    d_t = nc.alloc_sbuf_tensor("d_t", [G, 1], f32).ap()
    out_sb = nc.alloc_sbuf_tensor("out_sb", [G, 128], f32).ap()
    warm = nc.alloc_sbuf_tensor("warm", [G, 1], f32).ap()

    hs_view = a_sb[0:G, WA - 1 : WA]  # per-core h slice, base partition 0

    with (
        nc.semaphore() as s_idx,
        nc.semaphore() as s_in,
        nc.semaphore() as s_gx,
        nc.semaphore() as s_gx2,
        nc.semaphore() as s_v,
        nc.semaphore() as s_ve,
        nc.semaphore() as s_s,
        nc.semaphore() as s_out,
        nc.Block() as block,
    ):

        @block.sync
        def _(sync):
            sync.dma_start(idx_sb[:], idx_d[:]).then_inc(s_idx, 16)
            sync.dma_start(a_sb[:], a_d[:]).then_inc(s_in, 16)
            sync.dma_start(bh_sb[:], bh_d[:]).then_inc(s_in, 16)
            sync.wait_ge(s_v, 3)
            # No completion wait: engines halt after issue; the DMA lands
            # during the exit barrier / teardown, long before host readback.
            sync.dma_start(out_d[:], out_sb[:]).then_inc(s_out, 16)

        @block.gpsimd
        def _(gpsimd):
            gpsimd.wait_ge(s_idx, 16)
            # NOTE: splitting this into two indirect DMAs hangs on HW (works in
            # CoreSim and compiles) — keep a single gather.
            gpsimd.indirect_dma_start(
                out=bx_sb[:],
                out_offset=None,
                in_=table[:],
                in_offset=bass.IndirectOffsetOnAxis(ap=idx_sb[:, :1], axis=0),
            ).then_inc(s_gx, 16)

        @block.vector
        def _(vector):
            vector.wait_ge(s_in, 32)
            # h-side contraction + biases (ones columns): gh = sum(A_h * Bh)
            vector.scalar_tensor_tensor(
                out=s2[:],
                in0=a_sb[:, HID : HID + HID + 2],
                scalar=1.0,
                in1=bh_sb[:],
                op0=mybir.AluOpType.mult,
                op1=mybir.AluOpType.mult,
                accum_out=gh[:],
            ).then_inc(s_v, 1)
            vector.wait_ge(s_v, 1)  # sem edge for gh (same-engine RAW)
            vector.tensor_copy(out=ghn0[:], in_=gh[3 * G : 4 * G, :1]).then_inc(
                s_ve, 1
            )
            vector.wait_ge(s_gx, 16)
            # x-side contraction (r, i_n, z rows)
            vector.scalar_tensor_tensor(
                out=s1[:],
                in0=a_sb[0:GP, 0:HID],
                scalar=1.0,
                in1=bx_sb[:],
                op0=mybir.AluOpType.mult,
                op1=mybir.AluOpType.mult,
                accum_out=gx[:],
            ).then_inc(s_v, 1)
            vector.wait_ge(s_s, 4)  # n ready
            vector.tensor_tensor(
                out=d_t[:], in0=hs_view, in1=n_t[:], op=mybir.AluOpType.subtract
            ).then_inc(s_ve, 1)
            vector.wait_ge(s_s, 5)  # z_t ready
            vector.wait_ge(s_ve, 2)  # sem edge for d_t (same-engine RAW)
            vector.scalar_tensor_tensor(
                out=out_sb[:],
                in0=d_t[:, :1].to_broadcast([G, 128]),
                scalar=z_t[:, :1],
                in1=n_t[:, :1].to_broadcast([G, 128]),
                op0=mybir.AluOpType.mult,
                op1=mybir.AluOpType.add,
            ).then_inc(s_v, 1)

        @block.scalar
        def _(scalar):
            # Warm the ACT function table while DMAs/gather are in flight —
            # otherwise a ~1.3us ACT_TABLE_LOAD lands on the critical path.
            const0 = nc.const_aps.aps[(f32, 0.0)]
            scalar.activation(
                warm[:], const0[0:G, :1], mybir.ActivationFunctionType.Sigmoid
            )
            scalar.wait_ge(s_ve, 1)  # ghn0 ready
            scalar.wait_ge(s_v, 2)  # gh and gx[0:64] ready
            scalar.activation(
                r_t[:],
                gx[0:G, :1],
                mybir.ActivationFunctionType.Sigmoid,
                bias=gh[0:G, :1],
            ).then_inc(s_s, 1)
            scalar.wait_ge(s_s, 1)  # sem edge for r_t (same-engine RAW)
            # p = ghn0 * r   (ACT multiply: Copy with per-partition scale)
            scalar.activation(
                p_t[:],
                ghn0[:],
                mybir.ActivationFunctionType.Copy,
                scale=r_t[:, :1],
            ).then_inc(s_s, 1)
            scalar.activation(
                in0_t[:],
                gx[G : 2 * G, :1],
                mybir.ActivationFunctionType.Identity,
                bias=gh[G : 2 * G, :1],
            ).then_inc(s_s, 1)
            scalar.wait_ge(s_s, 3)  # sem edges for p_t, in0_t
            scalar.activation(
                n_t[:],
                p_t[:],
                mybir.ActivationFunctionType.Tanh,
                bias=in0_t[:, :1],
            ).then_inc(s_s, 1)
            scalar.activation(
                z_t[:],
                gx[2 * G : 3 * G, :1],
                mybir.ActivationFunctionType.Sigmoid,
                bias=gh[2 * G : 3 * G, :1],
            ).then_inc(s_s, 1)

    return nc


def shard_inputs(
    input, hidden, embedding, w_ih, w_hh, b_ih, b_hh
) -> list[dict[str, np.ndarray]]:
    """Host-side marshaling: slice/replicate full inputs into per-core maps."""
    idx = int(np.asarray(input).reshape(-1)[0])
    h = np.asarray(hidden, dtype=np.float32).reshape(HID)
    table = np.ascontiguousarray(np.asarray(embedding, dtype=np.float32))
    w_ih = np.asarray(w_ih, dtype=np.float32)
    w_hh = np.asarray(w_hh, dtype=np.float32)
    b_ih = np.asarray(b_ih, dtype=np.float32)
    b_hh = np.asarray(b_hh, dtype=np.float32)

    idx_arr = np.full((3 * G, 1), idx, dtype=np.int32)
    bh = np.empty((128, HID + 2), dtype=np.float32)
    bh[:, 0:HID] = h[None, :]
    bh[:, HID:] = 1.0

    in_maps = []
    for k in range(NCORES):
        lo = G * k
        a = np.zeros((128, WA), dtype=np.float32)
        # r rows
        a[0:G, 0:HID] = w_ih[lo : lo + G]
        a[0:G, HID : 2 * HID] = w_hh[lo : lo + G]
        a[0:G, 2 * HID] = b_ih[lo : lo + G]
        a[0:G, 2 * HID + 1] = b_hh[lo : lo + G]
        # i_n rows (x side only)
        a[G : 2 * G, 0:HID] = w_ih[2 * HID + lo : 2 * HID + lo + G]
        a[G : 2 * G, 2 * HID] = b_ih[2 * HID + lo : 2 * HID + lo + G]
        # z rows
        a[2 * G : 3 * G, 0:HID] = w_ih[HID + lo : HID + lo + G]
        a[2 * G : 3 * G, HID : 2 * HID] = w_hh[HID + lo : HID + lo + G]
        a[2 * G : 3 * G, 2 * HID] = b_ih[HID + lo : HID + lo + G]
        a[2 * G : 3 * G, 2 * HID + 1] = b_hh[HID + lo : HID + lo + G]
        # h_n rows (h side only)
        a[3 * G : 4 * G, HID : 2 * HID] = w_hh[2 * HID + lo : 2 * HID + lo + G]
        a[3 * G : 4 * G, 2 * HID] = b_hh[2 * HID + lo : 2 * HID + lo + G]
        # hs column (h slice for the final blend)
        a[0:G, 2 * HID + 2] = h[lo : lo + G]

        in_maps.append(
            {"table": table, "a_mat": a, "bh_mat": bh, "idx": idx_arr}
        )
    return in_maps


def unshard_output(results: list[dict[str, np.ndarray]]):
    h_new = np.concatenate(
        [np.asarray(results[k]["out"]).reshape(G, -1)[:, 0] for k in range(NCORES)]
    ).astype(np.float32)
    out = h_new.reshape(1, 1, HID)
    return out, out


def _get_program():
    global _cached
    if _cached is None:
        _cached = build_program()
    return _cached


def kernel(**inputs):
    from concourse.bass_utils import run_bass_kernel_spmd

    nc = _get_program()
    in_maps = shard_inputs(**inputs)
    res = run_bass_kernel_spmd(nc, in_maps, core_ids=list(range(NCORES)))
    return unshard_output(res.results)


def run_traced(**inputs):
    """Like kernel() but with NTFF tracing; returns (output, BassKernelResults)."""
    from concourse.bass_utils import run_bass_kernel_spmd

    nc = _get_program()
    in_maps = shard_inputs(**inputs)
    res = run_bass_kernel_spmd(nc, in_maps, core_ids=list(range(NCORES)), trace=True)
    return unshard_output(res.results), res
